# revision 28
# baseline (speedup 1.0000x reference)
"""Causal self-attention (B=4, T=1024, D=2048, H=16) on 8 trn2 NeuronCores.

Sharding: data-parallel over batch (4) x tensor-parallel over heads (2).
Core c handles batch b = c//2, head-half hh = c%2 (heads hh*8 .. hh*8+8).

Per-core plan (all matmuls float32r, fp32 PSUM accumulation):
  qk(head0) first -- PE warms up while the 8MB xT stream lands.
  v [t, c] as one dense phase (wv prefetched during qk0).
  Per head h: scores sT [tk,tq] (causal blocks only) -> exp via ACT ->
    row-sum gemm (single stationary ones vector, before yp so the
    reciprocal overlaps the y accumulation) -> y gemm -> normalize.
  Pairwise AllGather of TWO heads at a time (4 collectives), overlapped
  with later heads' compute.
  proj streams y chunks from the AllGather buffers in arrival order
  (wp rows host-permuted to match); first chunks prefetched during the
  last heads via a right-side SBUF pool; PSUM evacuation interleaved
  with the last contraction chunk.
Host side: slice/transpose inputs per core, concat outputs (pure gather).
"""

from contextlib import ExitStack

import numpy as np

import concourse.bass as bass
import concourse.mybir as mybir
import concourse.tile as tile
from concourse import bacc
from concourse.bass_utils import run_bass_kernel_spmd

B, T, D = 4, 1024, 2048
H, DH = 16, 128
N_CORES = 8
TP = 2                      # head-halves per batch
HPC = H // TP               # heads per core = 8
CPC = HPC * DH              # channels per core = 1024
KC = D // 128               # contraction chunks = 16
SCALE = 1.0 / float(np.sqrt(DH))

F32 = mybir.dt.float32
F32R = mybir.dt.float32r
BF16 = mybir.dt.bfloat16

PAIRS = [[2 * i, 2 * i + 1] for i in range(B)]

# AllGather groups of heads: small trailing groups so the last collective
# lands before the projection needs its chunks
AGS = [(0, 1), (2, 3), (4, 5), (6,), (7,)]
AG_OF_HEAD = {h: gi for gi, g in enumerate(AGS) for h in g}
# proj contraction chunk order = AllGather arrival order
CHUNKS = [(gi, r, si) for gi, g in enumerate(AGS)
          for r in range(TP) for si in range(len(g))]


def _score_chunks(w):
    """Split width w into matmul chunks, avoiding <256 when possible."""
    out = []
    off = 0
    while w > 0:
        if 512 < w < 768:
            cw = w // 2            # e.g. 640 -> 320+320 (both >=256)
        else:
            cw = min(512, w)
        out.append((off, cw))
        off += cw
        w -= cw
    return out


def build_kernel():
    nc = bacc.Bacc("TRN2", target_bir_lowering=False, debug=False,
                   num_devices=N_CORES)

    xT_ap = nc.dram_tensor("xT", [D, T], BF16, kind="ExternalInput").ap()
    # wq/wk are HOST-PRE-TRANSPOSED: row 128h+p, col 128k+j holds
    # w[128k+p, 128h+j] so each head's stationary tile loads contiguously
    wq_ap = nc.dram_tensor("wq", [CPC, D], BF16, kind="ExternalInput").ap()
    wk_ap = nc.dram_tensor("wk", [CPC, D], BF16, kind="ExternalInput").ap()
    wv_ap = nc.dram_tensor("wv", [D, CPC], BF16, kind="ExternalInput").ap()
    # wp rows are HOST-PERMUTED into AllGather arrival order:
    # chunk kk = (p, r, hp) -> original row block 1024*r + 128*(2*p+hp)
    wp_ap = nc.dram_tensor("wp", [D, CPC], BF16, kind="ExternalInput").ap()
    maskT_ap = nc.dram_tensor("maskT", [128, 128], BF16, kind="ExternalInput").ap()
    out_ap = nc.dram_tensor("out", [T, CPC], F32, kind="ExternalOutput").ap()

    with tile.TileContext(nc) as tc:
        _body(nc, tc, xT_ap, wq_ap, wk_ap, wv_ap, wp_ap, maskT_ap, out_ap)
    nc.compile()
    return nc


def _body(nc, tc, xT_ap, wq_ap, wk_ap, wv_ap, wp_ap, maskT_ap, out_ap):
    Exp = mybir.ActivationFunctionType.Exp
    mult = mybir.AluOpType.mult
    from concourse.dve_ops import RECIPROCAL_APPROX_NR

    s0 = ExitStack()
    with s0:
        const = s0.enter_context(tc.tile_pool(name="const", bufs=1))
        dram = s0.enter_context(tc.tile_pool(name="dramp", bufs=B, space="DRAM"))
        # right-side pool: proj-input prefetch issued during the last heads
        pre = s0.enter_context(tc.tile_pool(name="pre", bufs=4, side="right"))

        maskT = const.tile([128, 128], BF16, tag="maskT")
        nc.sync.dma_start(out=maskT, in_=maskT_ap)
        ones_f32 = const.tile([128, 1], F32, tag="ones_f32")
        nc.vector.memset(ones_f32, 1.0)
        ones_col = const.tile([128, 1], BF16, tag="ones_col")
        nc.scalar.copy(out=ones_col, in_=ones_f32)
        # warm the ACT exp table set during the input-DMA wall so head 0's
        # first real exp doesn't pay the ~2.7us table load
        exp_warm = const.tile([1, 1], F32, tag="exp_warm")
        nc.scalar.activation(out=exp_warm, in_=ones_f32[0:1, 0:1],
                             func=Exp, scale=1.0)

        # AllGather staging, one buffer pair per head group
        yt_pair = [dram.tile([len(g), 128, T], BF16, tag=f"ytp{gi}",
                             name=f"ytp{gi}")
                   for gi, g in enumerate(AGS)]
        yt_all = [dram.tile([TP, len(g), 128, T], BF16, tag=f"yta{gi}",
                            name=f"yta{gi}")
                  for gi, g in enumerate(AGS)]

        py_tiles = {}
        wp_tiles = {}

        def py_load(kk, pool):
            gi, r, si = CHUNKS[kk]
            t2 = pool.tile([128, T], BF16, tag="py", name=f"py{kk}")
            nc.sync.dma_start(out=t2, in_=yt_all[gi][r, si])
            py_tiles[kk] = t2

        def wp_load(cc, kk, pool):
            wt = pool.tile([128, 512], BF16, tag="wp", name=f"wp{cc}_{kk}")
            nc.scalar.dma_start(
                out=wt, in_=wp_ap[128 * kk:128 * (kk + 1),
                                  512 * cc:512 * (cc + 1)])
            wp_tiles[(cc, kk)] = wt

        s1 = ExitStack()
        with s1:
            xa = s1.enter_context(tc.tile_pool(name="xa", bufs=KC))
            vvp = s1.enter_context(tc.tile_pool(name="vvp", bufs=8))
            wqk = s1.enter_context(tc.tile_pool(name="wqk", bufs=2))
            qkp = s1.enter_context(tc.tile_pool(name="qkp", bufs=3))
            ptp = s1.enter_context(tc.tile_pool(name="pt", bufs=1))
            yt_pool = s1.enter_context(tc.tile_pool(name="yt", bufs=2))
            asm = s1.enter_context(tc.tile_pool(name="att_sm", bufs=1))
            wvp = s1.enter_context(tc.tile_pool(name="wv", bufs=3))

            def load_wqk(h2, split=1):
                # wq on the sync HWDGE ring, wk on the scalar ring so the
                # two 1MB streams land in parallel; head 0 is split into
                # halves so its k-loop can start sooner
                tiles = []
                kc_s = KC // split
                for (w_ap, nm, eng) in ((wq_ap, "q", nc.sync),
                                        (wk_ap, "k", nc.scalar)):
                    wt = wqk.tile([128, KC, 128], BF16, tag="wqk",
                                  name=f"w{nm}{h2}")
                    for s in range(split):
                        eng.dma_start(
                            out=wt[:, kc_s * s:kc_s * (s + 1), :],
                            in_=w_ap[128 * h2:128 * (h2 + 1),
                                     128 * kc_s * s:128 * kc_s * (s + 1)
                                     ].rearrange("p (k j) -> p k j", k=kc_s))
                    tiles.append(wt)
                return tiles

            # input streams: wqk(head0) first; xT chunks ride the SWDGE
            # (gpsimd) rings so they don't serialize behind the HWDGE queues
            wts = {0: load_wqk(0, split=4)}
            xts = []
            for k in range(KC):
                xt = xa.tile([128, T], BF16, tag="xT", name=f"xt{k}")
                nc.gpsimd.dma_start(out=xt,
                                    in_=xT_ap[128 * k:128 * (k + 1), :])
                xts.append(xt)
            vv = [vvp.tile([128, CPC], BF16, tag="vv", name=f"vv{j}")
                  for j in range(8)]

            def qk_gemm(h, pool):
                """qT/kT [dh, T] for head h; weight loaded once per (wi,k)."""
                wts_cur = wts.pop(h)
                qkT = []
                for wi, nm in ((0, "q"), (1, "k")):
                    outT = qkp.tile([128, T], BF16, tag="qkT",
                                    name=f"{nm}T{h}")
                    qkT.append(outT)
                    wt = wts_cur[wi]
                    ps0 = pool.tile([128, 512], F32, tag="pqk",
                                    name=f"pqk{h}{nm}0")
                    ps1 = pool.tile([128, 512], F32, tag="pqk",
                                    name=f"pqk{h}{nm}1")
                    for k in range(KC):
                        nc.tensor.matmul(ps0, wt[:, k, :], xts[k][:, 0:512],
                                         start=(k == 0), stop=(k == KC - 1))
                        nc.tensor.matmul(ps1, wt[:, k, :], xts[k][:, 512:1024],
                                         start=(k == 0), stop=(k == KC - 1))
                    nc.vector.tensor_copy(out=outT[:, 0:512], in_=ps0)
                    nc.vector.tensor_copy(out=outT[:, 512:1024], in_=ps1)
                return qkT

            def v_half(ch, pv):
                """v[t, c] for columns 512*ch .. 512*ch+512."""
                ps = [pv.tile([128, 512], F32, tag="pv", name=f"pv{ch}_{i}")
                      for i in range(8)]
                for k in range(KC):
                    wt = wvp.tile([128, 512], BF16, tag="wv", name=f"wv{ch}_{k}")
                    nc.scalar.dma_start(
                        out=wt,
                        in_=wv_ap[128 * k:128 * (k + 1),
                                  512 * ch:512 * (ch + 1)])
                    for tch in range(8):
                        nc.tensor.matmul(
                            ps[tch], xts[k][:, 128 * tch:128 * (tch + 1)], wt,
                            start=(k == 0), stop=(k == KC - 1))
                for tch in range(8):
                    nc.vector.tensor_copy(
                        out=vv[tch][:, 512 * ch:512 * (ch + 1)], in_=ps[tch])

            def attn_scores(h, qTh, kTh, pss):
                """scores -> exp/mask into SBUF probability tiles."""
                pts = []
                for j in range(8):
                    w = T - 128 * j
                    pt = ptp.tile([128, w], BF16, tag=f"pT{j}",
                                  name=f"pT{h}_{j}")
                    pts.append(pt)
                    for (off, cw) in _score_chunks(w):
                        sp = pss.tile([128, 512], F32, tag="sT")
                        nc.tensor.matmul(
                            sp[:, :cw], kTh[:, 128 * j:128 * (j + 1)],
                            qTh[:, 128 * j + off:128 * j + off + cw],
                            start=True, stop=True)
                        nc.scalar.activation(
                            out=pt[:, off:off + cw], in_=sp[:, :cw],
                            func=Exp, scale=SCALE)
                    nc.vector.tensor_tensor(
                        out=pt[:, 0:128], in0=pt[:, 0:128], in1=maskT, op=mult)
                return pts

            def attn_finish(h, pts, psy, psr):
                """row sums -> reciprocal -> y accumulation -> normalize."""
                rp = [psr.tile([1, 512], F32, tag="rp", name=f"rp{h}_{g}")
                      for g in range(2)]
                started = [False, False]
                for j in range(8):
                    for g in range(2):
                        tq0 = 512 * g
                        lo = max(tq0, 128 * j)
                        w = tq0 + 512 - lo
                        if w <= 0:
                            continue
                        last = (j == 7) or (g == 0 and j == 3)
                        nc.tensor.matmul(
                            rp[g][:, lo - tq0:lo - tq0 + w], ones_col,
                            pts[j][:, lo - 128 * j:lo - 128 * j + w],
                            start=(not started[g]), stop=last)
                        started[g] = True

                # softmax denominators: copy -> recip(+NR) -> broadcast
                r_sb = asm.tile([1, T], F32, tag="r_sb")
                nc.vector.tensor_copy(out=r_sb[:, 0:512], in_=rp[0])
                nc.vector.tensor_copy(out=r_sb[:, 512:1024], in_=rp[1])
                rec1 = asm.tile([1, T], F32, tag="rec1")
                nc.vector.reciprocal_approx_fast(out=rec1, in_=r_sb)
                nc.vector._custom_dve(RECIPROCAL_APPROX_NR, out=rec1,
                                      in0=r_sb, in1=rec1, s0=2.0)
                rec = asm.tile([128, T], F32, tag="rec")
                nc.gpsimd.partition_broadcast(rec, rec1)

                # y accumulation: v-block stationary once per j
                yp = [psy.tile([128, 512], F32, tag="yp", name=f"yp{h}_{g}")
                      for g in range(2)]
                ystarted = [False, False]
                for j in range(8):
                    vblk = vv[j][:, 128 * h:128 * (h + 1)]
                    for g in range(2):
                        tq0 = 512 * g
                        lo = max(tq0, 128 * j)
                        w = tq0 + 512 - lo
                        if w <= 0:
                            continue
                        last = (j == 7) or (g == 0 and j == 3)
                        nc.tensor.matmul(
                            yp[g][:, lo - tq0:lo - tq0 + w], vblk,
                            pts[j][:, lo - 128 * j:lo - 128 * j + w],
                            start=(not ystarted[g]), stop=last)
                        ystarted[g] = True

                yt = yt_pool.tile([128, T], BF16, tag="yT", name=f"yt{h}")
                for g in range(2):
                    nc.vector.tensor_tensor(
                        out=yt[:, 512 * g:512 * (g + 1)], in0=yp[g],
                        in1=rec[:, 512 * g:512 * (g + 1)], op=mult)
                # stage for the pairwise AllGather of this head's group
                gi = AG_OF_HEAD[h]
                si = AGS[gi].index(h)
                nc.sync.dma_start(out=yt_pair[gi][si], in_=yt)
                if h == AGS[gi][-1]:
                    nc.gpsimd.collective_compute(
                        "AllGather", mybir.AluOpType.bypass,
                        replica_groups=PAIRS,
                        ins=[yt_pair[gi].opt()],
                        outs=[yt_all[gi].opt()])

            # ---- phase 1: qk(head0) while xT streams in ----
            with tc.tile_pool(name="pqk0", bufs=2, space="PSUM") as pqk0:
                qkT0 = qk_gemm(0, pqk0)
                wts[1] = load_wqk(1)

            # ---- phase 2: v (wv prefetched during phase 1) ----
            with tc.tile_pool(name="pv", bufs=8, space="PSUM") as pv:
                v_half(0, pv)
                v_half(1, pv)

            # ---- phase 3: heads ----
            s1c = ExitStack()
            with s1c:
                pa = s1c.enter_context(
                    tc.tile_pool(name="pa", bufs=2, space="PSUM"))
                pss = s1c.enter_context(
                    tc.tile_pool(name="ps_s", bufs=2, space="PSUM"))
                psy = s1c.enter_context(
                    tc.tile_pool(name="ps_y", bufs=2, space="PSUM"))
                psr = s1c.enter_context(
                    tc.tile_pool(name="ps_r", bufs=2, space="PSUM"))

                # steady state per head: scores_h -> qk_{h+1} (PE busy
                # while ACT streams the exps) -> rowsums/y of head h
                pts_cur = attn_scores(0, qkT0[0], qkT0[1], pss)
                qk_next = None
                for h in range(HPC):
                    if h + 1 < HPC:
                        qk_next = qk_gemm(h + 1, pa)
                        if h + 2 < HPC:
                            wts[h + 2] = load_wqk(h + 2)
                    # prefetch proj inputs late in the head loop
                    if h == 6:
                        for kk in range(2):
                            py_load(kk, pre)
                        for kk in range(2):
                            wp_load(0, kk, pre)
                    elif h == 7:
                        for kk in range(2, 4):
                            py_load(kk, pre)
                        for kk in range(2, 4):
                            wp_load(0, kk, pre)
                    attn_finish(h, pts_cur, psy, psr)
                    if h + 1 < HPC:
                        pts_cur = attn_scores(h + 1, qk_next[0], qk_next[1],
                                              pss)

        # ---- phase 4: output projection (chunks in AG arrival order) ----
        s2 = ExitStack()
        with s2:
            pyp = s2.enter_context(tc.tile_pool(name="py", bufs=12))
            wpp = s2.enter_context(tc.tile_pool(name="wp", bufs=8))
            osb = s2.enter_context(tc.tile_pool(name="out_sb", bufs=4))
            pso = s2.enter_context(
                tc.tile_pool(name="ps_o", bufs=8, space="PSUM"))

            for kk in range(4, 2 * HPC):
                py_load(kk, pyp)
            for kk in range(4, 2 * HPC):
                wp_load(0, kk, wpp)
            for kk in range(2 * HPC):
                wp_load(1, kk, wpp)

            for cc in range(2):
                ps = [pso.tile([128, 512], F32, tag="po",
                               name=f"po{cc}_{m}") for m in range(8)]
                last_kk = 2 * HPC - 1
                for kk in range(last_kk):
                    yk = py_tiles[kk]
                    wt = wp_tiles[(cc, kk)]
                    for m in range(8):
                        nc.tensor.matmul(
                            ps[m], yk[:, 128 * m:128 * (m + 1)], wt,
                            start=(kk == 0), stop=False)
                # last chunk interleaved with evacuation so the next cc
                # pass's PSUM banks free up m-by-m
                yk = py_tiles[last_kk]
                wt = wp_tiles[(cc, last_kk)]
                for m in range(8):
                    nc.tensor.matmul(
                        ps[m], yk[:, 128 * m:128 * (m + 1)], wt,
                        start=False, stop=True)
                    ot = osb.tile([128, 512], F32, tag="ot")
                    nc.scalar.copy(out=ot, in_=ps[m])
                    nc.sync.dma_start(
                        out=out_ap[128 * m:128 * (m + 1),
                                   512 * cc:512 * (cc + 1)],
                        in_=ot)


_NC_CACHE = None


def _get_nc():
    global _NC_CACHE
    if _NC_CACHE is None:
        _NC_CACHE = build_kernel()
    return _NC_CACHE


def kernel(x, w_qkv, w_proj, _trace=False, _trace_kwargs=None):
    x = np.asarray(x, dtype=np.float32)
    w_qkv = np.asarray(w_qkv, dtype=np.float32)
    w_proj = np.asarray(w_proj, dtype=np.float32)

    import ml_dtypes
    bf16 = ml_dtypes.bfloat16
    maskT = np.triu(np.ones((128, 128), dtype=np.float32)).astype(bf16)

    # proj-weight row permutation: AllGather arrival order
    # chunk kk = (gi, r, si) -> original rows 1024*r + 128*AGS[gi][si]
    perm = np.concatenate([
        np.arange(1024 * r + 128 * AGS[gi][si],
                  1024 * r + 128 * AGS[gi][si] + 128)
        for (gi, r, si) in CHUNKS])

    def qk_layout(w):
        # [D, CPC] -> rows 128h+p, cols 128k+j hold w[128k+p, 128h+j]
        t = w.reshape(KC, 128, HPC, 128)
        return np.ascontiguousarray(
            t.transpose(2, 1, 0, 3).reshape(CPC, D))

    in_maps = []
    for c in range(N_CORES):
        b, hh = c // TP, c % TP
        cols = slice(hh * CPC, (hh + 1) * CPC)
        in_maps.append({
            "xT": np.ascontiguousarray(x[b].T).astype(bf16),
            "wq": qk_layout(w_qkv[:, :D][:, cols]).astype(bf16),
            "wk": qk_layout(w_qkv[:, D:2 * D][:, cols]).astype(bf16),
            "wv": np.ascontiguousarray(w_qkv[:, 2 * D:][:, cols]).astype(bf16),
            "wp": np.ascontiguousarray(w_proj[perm][:, cols]).astype(bf16),
            "maskT": maskT,
        })

    nc = _get_nc()
    res = run_bass_kernel_spmd(nc, in_maps, list(range(N_CORES)),
                               trace=_trace, **(_trace_kwargs or {}))

    out = np.empty((B, T, D), dtype=np.float32)
    for c in range(N_CORES):
        b, hh = c // TP, c % TP
        out[b, :, hh * CPC:(hh + 1) * CPC] = res.results[c]["out"]
    if _trace:
        return out, res
    return out


# revision 29
# speedup vs baseline: 1.0037x; 1.0037x over previous
"""Causal self-attention (B=4, T=1024, D=2048, H=16) on 8 trn2 NeuronCores.

Sharding: data-parallel over batch (4) x tensor-parallel over heads (2).
Core c handles batch b = c//2, head-half hh = c%2 (heads hh*8 .. hh*8+8).

Per-core plan (all matmuls float32r, fp32 PSUM accumulation):
  qk(head0) first -- PE warms up while the 8MB xT stream lands.
  v [t, c] as one dense phase (wv prefetched during qk0).
  Per head h: scores sT [tk,tq] (causal blocks only) -> exp via ACT ->
    row-sum gemm (single stationary ones vector, before yp so the
    reciprocal overlaps the y accumulation) -> y gemm -> normalize.
  Pairwise AllGather of TWO heads at a time (4 collectives), overlapped
  with later heads' compute.
  proj streams y chunks from the AllGather buffers in arrival order
  (wp rows host-permuted to match); first chunks prefetched during the
  last heads via a right-side SBUF pool; PSUM evacuation interleaved
  with the last contraction chunk.
Host side: slice/transpose inputs per core, concat outputs (pure gather).
"""

from contextlib import ExitStack

import numpy as np

import concourse.bass as bass
import concourse.mybir as mybir
import concourse.tile as tile
from concourse import bacc
from concourse.bass_utils import run_bass_kernel_spmd

B, T, D = 4, 1024, 2048
H, DH = 16, 128
N_CORES = 8
TP = 2                      # head-halves per batch
HPC = H // TP               # heads per core = 8
CPC = HPC * DH              # channels per core = 1024
KC = D // 128               # contraction chunks = 16
SCALE = 1.0 / float(np.sqrt(DH))

F32 = mybir.dt.float32
F32R = mybir.dt.float32r
BF16 = mybir.dt.bfloat16

PAIRS = [[2 * i, 2 * i + 1] for i in range(B)]

# AllGather groups of heads: small trailing groups so the last collective
# lands before the projection needs its chunks
AGS = [(0, 1), (2, 3), (4, 5), (6,), (7,)]
AG_OF_HEAD = {h: gi for gi, g in enumerate(AGS) for h in g}
# proj contraction chunk order = AllGather arrival order
CHUNKS = [(gi, r, si) for gi, g in enumerate(AGS)
          for r in range(TP) for si in range(len(g))]


def _score_chunks(w):
    """Split width w into matmul chunks, avoiding <256 when possible."""
    out = []
    off = 0
    while w > 0:
        if 512 < w < 768:
            cw = w // 2            # e.g. 640 -> 320+320 (both >=256)
        else:
            cw = min(512, w)
        out.append((off, cw))
        off += cw
        w -= cw
    return out


def build_kernel():
    nc = bacc.Bacc("TRN2", target_bir_lowering=False, debug=False,
                   num_devices=N_CORES)

    xT_ap = nc.dram_tensor("xT", [D, T], BF16, kind="ExternalInput").ap()
    # wq/wk are HOST-PRE-TRANSPOSED: row 128h+p, col 128k+j holds
    # w[128k+p, 128h+j] so each head's stationary tile loads contiguously
    wq_ap = nc.dram_tensor("wq", [CPC, D], BF16, kind="ExternalInput").ap()
    wk_ap = nc.dram_tensor("wk", [CPC, D], BF16, kind="ExternalInput").ap()
    wv_ap = nc.dram_tensor("wv", [D, CPC], BF16, kind="ExternalInput").ap()
    # wp rows are HOST-PERMUTED into AllGather arrival order:
    # chunk kk = (p, r, hp) -> original row block 1024*r + 128*(2*p+hp)
    wp_ap = nc.dram_tensor("wp", [D, CPC], BF16, kind="ExternalInput").ap()
    maskT_ap = nc.dram_tensor("maskT", [128, 128], BF16, kind="ExternalInput").ap()
    out_ap = nc.dram_tensor("out", [T, CPC], F32, kind="ExternalOutput").ap()

    with tile.TileContext(nc) as tc:
        _body(nc, tc, xT_ap, wq_ap, wk_ap, wv_ap, wp_ap, maskT_ap, out_ap)
    nc.compile()
    return nc


def _body(nc, tc, xT_ap, wq_ap, wk_ap, wv_ap, wp_ap, maskT_ap, out_ap):
    Exp = mybir.ActivationFunctionType.Exp
    mult = mybir.AluOpType.mult
    from concourse.dve_ops import RECIPROCAL_APPROX_NR

    s0 = ExitStack()
    with s0:
        const = s0.enter_context(tc.tile_pool(name="const", bufs=1))
        dram = s0.enter_context(tc.tile_pool(name="dramp", bufs=B, space="DRAM"))
        # right-side pool: proj-input prefetch issued during the last heads
        pre = s0.enter_context(tc.tile_pool(name="pre", bufs=4, side="right"))

        maskT = const.tile([128, 128], BF16, tag="maskT")
        nc.sync.dma_start(out=maskT, in_=maskT_ap)
        ones_f32 = const.tile([128, 1], F32, tag="ones_f32")
        nc.vector.memset(ones_f32, 1.0)
        ones_col = const.tile([128, 1], BF16, tag="ones_col")
        nc.scalar.copy(out=ones_col, in_=ones_f32)
        # warm the ACT exp table set during the input-DMA wall so head 0's
        # first real exp doesn't pay the ~2.7us table load
        exp_warm = const.tile([1, 1], F32, tag="exp_warm")
        nc.scalar.activation(out=exp_warm, in_=ones_f32[0:1, 0:1],
                             func=Exp, scale=1.0)

        # AllGather staging, one buffer pair per head group
        yt_pair = [dram.tile([len(g), 128, T], BF16, tag=f"ytp{gi}",
                             name=f"ytp{gi}")
                   for gi, g in enumerate(AGS)]
        yt_all = [dram.tile([TP, len(g), 128, T], BF16, tag=f"yta{gi}",
                            name=f"yta{gi}")
                  for gi, g in enumerate(AGS)]

        py_tiles = {}
        wp_tiles = {}

        def py_load(kk, pool):
            gi, r, si = CHUNKS[kk]
            t2 = pool.tile([128, T], BF16, tag="py", name=f"py{kk}")
            nc.sync.dma_start(out=t2, in_=yt_all[gi][r, si])
            py_tiles[kk] = t2

        def wp_load(cc, kk, pool):
            wt = pool.tile([128, 512], BF16, tag="wp", name=f"wp{cc}_{kk}")
            nc.scalar.dma_start(
                out=wt, in_=wp_ap[128 * kk:128 * (kk + 1),
                                  512 * cc:512 * (cc + 1)])
            wp_tiles[(cc, kk)] = wt

        s1 = ExitStack()
        with s1:
            xa = s1.enter_context(tc.tile_pool(name="xa", bufs=KC))
            vvp = s1.enter_context(tc.tile_pool(name="vvp", bufs=8))
            wqk = s1.enter_context(tc.tile_pool(name="wqk", bufs=2))
            qkp = s1.enter_context(tc.tile_pool(name="qkp", bufs=3))
            ptp = s1.enter_context(tc.tile_pool(name="pt", bufs=1))
            yt_pool = s1.enter_context(tc.tile_pool(name="yt", bufs=2))
            asm = s1.enter_context(tc.tile_pool(name="att_sm", bufs=1))
            wvp = s1.enter_context(tc.tile_pool(name="wv", bufs=3))

            def load_wqk(h2, split=1):
                # wq on the sync HWDGE ring, wk on the scalar ring so the
                # two 1MB streams land in parallel; head 0 is split into
                # halves so its k-loop can start sooner
                tiles = []
                kc_s = KC // split
                for (w_ap, nm, eng) in ((wq_ap, "q", nc.sync),
                                        (wk_ap, "k", nc.scalar)):
                    wt = wqk.tile([128, KC, 128], BF16, tag="wqk",
                                  name=f"w{nm}{h2}")
                    for s in range(split):
                        eng.dma_start(
                            out=wt[:, kc_s * s:kc_s * (s + 1), :],
                            in_=w_ap[128 * h2:128 * (h2 + 1),
                                     128 * kc_s * s:128 * kc_s * (s + 1)
                                     ].rearrange("p (k j) -> p k j", k=kc_s))
                    tiles.append(wt)
                return tiles

            # input streams: wqk(head0) first; xT chunks ride the SWDGE
            # (gpsimd) rings so they don't serialize behind the HWDGE queues
            wts = {0: load_wqk(0, split=4)}
            xts = []
            for k in range(KC):
                xt = xa.tile([128, T], BF16, tag="xT", name=f"xt{k}")
                nc.gpsimd.dma_start(out=xt,
                                    in_=xT_ap[128 * k:128 * (k + 1), :])
                xts.append(xt)
            vv = [vvp.tile([128, CPC], BF16, tag="vv", name=f"vv{j}")
                  for j in range(8)]

            def qk_gemm(h, pool):
                """qT/kT [dh, T] for head h; weight loaded once per (wi,k)."""
                wts_cur = wts.pop(h)
                qkT = []
                for wi, nm in ((0, "q"), (1, "k")):
                    outT = qkp.tile([128, T], BF16, tag="qkT",
                                    name=f"{nm}T{h}")
                    qkT.append(outT)
                    wt = wts_cur[wi]
                    ps0 = pool.tile([128, 512], F32, tag="pqk",
                                    name=f"pqk{h}{nm}0")
                    ps1 = pool.tile([128, 512], F32, tag="pqk",
                                    name=f"pqk{h}{nm}1")
                    for k in range(KC):
                        nc.tensor.matmul(ps0, wt[:, k, :], xts[k][:, 0:512],
                                         start=(k == 0), stop=(k == KC - 1))
                        nc.tensor.matmul(ps1, wt[:, k, :], xts[k][:, 512:1024],
                                         start=(k == 0), stop=(k == KC - 1))
                    nc.scalar.copy(out=outT[:, 0:512], in_=ps0)
                    nc.scalar.copy(out=outT[:, 512:1024], in_=ps1)
                return qkT

            def v_half(ch, pv):
                """v[t, c] for columns 512*ch .. 512*ch+512."""
                ps = [pv.tile([128, 512], F32, tag="pv", name=f"pv{ch}_{i}")
                      for i in range(8)]
                for k in range(KC):
                    wt = wvp.tile([128, 512], BF16, tag="wv", name=f"wv{ch}_{k}")
                    nc.scalar.dma_start(
                        out=wt,
                        in_=wv_ap[128 * k:128 * (k + 1),
                                  512 * ch:512 * (ch + 1)])
                    for tch in range(8):
                        nc.tensor.matmul(
                            ps[tch], xts[k][:, 128 * tch:128 * (tch + 1)], wt,
                            start=(k == 0), stop=(k == KC - 1))
                for tch in range(8):
                    nc.vector.tensor_copy(
                        out=vv[tch][:, 512 * ch:512 * (ch + 1)], in_=ps[tch])

            def attn_scores(h, qTh, kTh, pss):
                """scores -> exp/mask into SBUF probability tiles."""
                pts = []
                for j in range(8):
                    w = T - 128 * j
                    pt = ptp.tile([128, w], BF16, tag=f"pT{j}",
                                  name=f"pT{h}_{j}")
                    pts.append(pt)
                    for (off, cw) in _score_chunks(w):
                        sp = pss.tile([128, 512], F32, tag="sT")
                        nc.tensor.matmul(
                            sp[:, :cw], kTh[:, 128 * j:128 * (j + 1)],
                            qTh[:, 128 * j + off:128 * j + off + cw],
                            start=True, stop=True)
                        nc.scalar.activation(
                            out=pt[:, off:off + cw], in_=sp[:, :cw],
                            func=Exp, scale=SCALE)
                    nc.vector.tensor_tensor(
                        out=pt[:, 0:128], in0=pt[:, 0:128], in1=maskT, op=mult)
                return pts

            def attn_finish(h, pts, psy, psr):
                """row sums -> reciprocal -> y accumulation -> normalize."""
                rp = [psr.tile([1, 512], F32, tag="rp", name=f"rp{h}_{g}")
                      for g in range(2)]
                started = [False, False]
                for j in range(8):
                    for g in range(2):
                        tq0 = 512 * g
                        lo = max(tq0, 128 * j)
                        w = tq0 + 512 - lo
                        if w <= 0:
                            continue
                        last = (j == 7) or (g == 0 and j == 3)
                        nc.tensor.matmul(
                            rp[g][:, lo - tq0:lo - tq0 + w], ones_col,
                            pts[j][:, lo - 128 * j:lo - 128 * j + w],
                            start=(not started[g]), stop=last)
                        started[g] = True

                # softmax denominators: copy -> recip(+NR) -> broadcast
                r_sb = asm.tile([1, T], F32, tag="r_sb")
                nc.vector.tensor_copy(out=r_sb[:, 0:512], in_=rp[0])
                nc.vector.tensor_copy(out=r_sb[:, 512:1024], in_=rp[1])
                rec1 = asm.tile([1, T], F32, tag="rec1")
                nc.vector.reciprocal_approx_fast(out=rec1, in_=r_sb)
                nc.vector._custom_dve(RECIPROCAL_APPROX_NR, out=rec1,
                                      in0=r_sb, in1=rec1, s0=2.0)
                rec = asm.tile([128, T], F32, tag="rec")
                nc.gpsimd.partition_broadcast(rec, rec1)

                # y accumulation: v-block stationary once per j
                yp = [psy.tile([128, 512], F32, tag="yp", name=f"yp{h}_{g}")
                      for g in range(2)]
                ystarted = [False, False]
                for j in range(8):
                    vblk = vv[j][:, 128 * h:128 * (h + 1)]
                    for g in range(2):
                        tq0 = 512 * g
                        lo = max(tq0, 128 * j)
                        w = tq0 + 512 - lo
                        if w <= 0:
                            continue
                        last = (j == 7) or (g == 0 and j == 3)
                        nc.tensor.matmul(
                            yp[g][:, lo - tq0:lo - tq0 + w], vblk,
                            pts[j][:, lo - 128 * j:lo - 128 * j + w],
                            start=(not ystarted[g]), stop=last)
                        ystarted[g] = True

                yt = yt_pool.tile([128, T], BF16, tag="yT", name=f"yt{h}")
                for g in range(2):
                    nc.vector.tensor_tensor(
                        out=yt[:, 512 * g:512 * (g + 1)], in0=yp[g],
                        in1=rec[:, 512 * g:512 * (g + 1)], op=mult)
                # stage for the pairwise AllGather of this head's group
                gi = AG_OF_HEAD[h]
                si = AGS[gi].index(h)
                nc.sync.dma_start(out=yt_pair[gi][si], in_=yt)
                if h == AGS[gi][-1]:
                    nc.gpsimd.collective_compute(
                        "AllGather", mybir.AluOpType.bypass,
                        replica_groups=PAIRS,
                        ins=[yt_pair[gi].opt()],
                        outs=[yt_all[gi].opt()])

            # ---- phase 1: qk(head0) while xT streams in ----
            with tc.tile_pool(name="pqk0", bufs=2, space="PSUM") as pqk0:
                qkT0 = qk_gemm(0, pqk0)
                wts[1] = load_wqk(1)

            # ---- phase 2: v (wv prefetched during phase 1) ----
            with tc.tile_pool(name="pv", bufs=8, space="PSUM") as pv:
                v_half(0, pv)
                v_half(1, pv)

            # ---- phase 3: heads ----
            s1c = ExitStack()
            with s1c:
                pa = s1c.enter_context(
                    tc.tile_pool(name="pa", bufs=2, space="PSUM"))
                pss = s1c.enter_context(
                    tc.tile_pool(name="ps_s", bufs=2, space="PSUM"))
                psy = s1c.enter_context(
                    tc.tile_pool(name="ps_y", bufs=2, space="PSUM"))
                psr = s1c.enter_context(
                    tc.tile_pool(name="ps_r", bufs=2, space="PSUM"))

                # steady state per head: scores_h -> qk_{h+1} (PE busy
                # while ACT streams the exps) -> rowsums/y of head h
                pts_cur = attn_scores(0, qkT0[0], qkT0[1], pss)
                qk_next = None
                for h in range(HPC):
                    if h + 1 < HPC:
                        qk_next = qk_gemm(h + 1, pa)
                        if h + 2 < HPC:
                            wts[h + 2] = load_wqk(h + 2)
                    # prefetch proj inputs late in the head loop
                    if h == 6:
                        for kk in range(2):
                            py_load(kk, pre)
                        for kk in range(2):
                            wp_load(0, kk, pre)
                    elif h == 7:
                        for kk in range(2, 4):
                            py_load(kk, pre)
                        for kk in range(2, 4):
                            wp_load(0, kk, pre)
                    attn_finish(h, pts_cur, psy, psr)
                    if h + 1 < HPC:
                        pts_cur = attn_scores(h + 1, qk_next[0], qk_next[1],
                                              pss)

        # ---- phase 4: output projection (chunks in AG arrival order) ----
        s2 = ExitStack()
        with s2:
            pyp = s2.enter_context(tc.tile_pool(name="py", bufs=12))
            wpp = s2.enter_context(tc.tile_pool(name="wp", bufs=8))
            osb = s2.enter_context(tc.tile_pool(name="out_sb", bufs=4))
            pso = s2.enter_context(
                tc.tile_pool(name="ps_o", bufs=8, space="PSUM"))

            for kk in range(4, 2 * HPC):
                py_load(kk, pyp)
            for kk in range(4, 2 * HPC):
                wp_load(0, kk, wpp)
            for kk in range(2 * HPC):
                wp_load(1, kk, wpp)

            for cc in range(2):
                ps = [pso.tile([128, 512], F32, tag="po",
                               name=f"po{cc}_{m}") for m in range(8)]
                last_kk = 2 * HPC - 1
                for kk in range(last_kk):
                    yk = py_tiles[kk]
                    wt = wp_tiles[(cc, kk)]
                    for m in range(8):
                        nc.tensor.matmul(
                            ps[m], yk[:, 128 * m:128 * (m + 1)], wt,
                            start=(kk == 0), stop=False)
                # last chunk interleaved with evacuation so the next cc
                # pass's PSUM banks free up m-by-m
                yk = py_tiles[last_kk]
                wt = wp_tiles[(cc, last_kk)]
                for m in range(8):
                    nc.tensor.matmul(
                        ps[m], yk[:, 128 * m:128 * (m + 1)], wt,
                        start=False, stop=True)
                    ot = osb.tile([128, 512], F32, tag="ot")
                    nc.scalar.copy(out=ot, in_=ps[m])
                    nc.sync.dma_start(
                        out=out_ap[128 * m:128 * (m + 1),
                                   512 * cc:512 * (cc + 1)],
                        in_=ot)


_NC_CACHE = None


def _get_nc():
    global _NC_CACHE
    if _NC_CACHE is None:
        _NC_CACHE = build_kernel()
    return _NC_CACHE


def kernel(x, w_qkv, w_proj, _trace=False, _trace_kwargs=None):
    x = np.asarray(x, dtype=np.float32)
    w_qkv = np.asarray(w_qkv, dtype=np.float32)
    w_proj = np.asarray(w_proj, dtype=np.float32)

    import ml_dtypes
    bf16 = ml_dtypes.bfloat16
    maskT = np.triu(np.ones((128, 128), dtype=np.float32)).astype(bf16)

    # proj-weight row permutation: AllGather arrival order
    # chunk kk = (gi, r, si) -> original rows 1024*r + 128*AGS[gi][si]
    perm = np.concatenate([
        np.arange(1024 * r + 128 * AGS[gi][si],
                  1024 * r + 128 * AGS[gi][si] + 128)
        for (gi, r, si) in CHUNKS])

    def qk_layout(w):
        # [D, CPC] -> rows 128h+p, cols 128k+j hold w[128k+p, 128h+j]
        t = w.reshape(KC, 128, HPC, 128)
        return np.ascontiguousarray(
            t.transpose(2, 1, 0, 3).reshape(CPC, D))

    in_maps = []
    for c in range(N_CORES):
        b, hh = c // TP, c % TP
        cols = slice(hh * CPC, (hh + 1) * CPC)
        in_maps.append({
            "xT": np.ascontiguousarray(x[b].T).astype(bf16),
            "wq": qk_layout(w_qkv[:, :D][:, cols]).astype(bf16),
            "wk": qk_layout(w_qkv[:, D:2 * D][:, cols]).astype(bf16),
            "wv": np.ascontiguousarray(w_qkv[:, 2 * D:][:, cols]).astype(bf16),
            "wp": np.ascontiguousarray(w_proj[perm][:, cols]).astype(bf16),
            "maskT": maskT,
        })

    nc = _get_nc()
    res = run_bass_kernel_spmd(nc, in_maps, list(range(N_CORES)),
                               trace=_trace, **(_trace_kwargs or {}))

    out = np.empty((B, T, D), dtype=np.float32)
    for c in range(N_CORES):
        b, hh = c // TP, c % TP
        out[b, :, hh * CPC:(hh + 1) * CPC] = res.results[c]["out"]
    if _trace:
        return out, res
    return out
